# revision 14
# baseline (speedup 1.0000x reference)
"""Trainium2 Bass kernel for a 2-layer GATv2 encoder + LayerNorm (ASTGATEncoder).

Strategy (8 NeuronCores, SPMD single NEFF):
  - Nodes are greedily balanced into (core, block) bins of <=128 dst nodes,
    equalizing in-edge counts. Each core owns 1/8 of the nodes (10 blocks).
  - Dense transforms: layer 1 replicated (xl1 for all nodes per core, xr1 for
    own nodes); layer 2: transposed h AllGathered per pair of blocks (half the
    bytes of xl2), xl2 for all gathered rows then computed locally, pipelined
    against the layer-1 edge loop.
  - Edge phase per block: edges sorted by dst slot, padded to CBLK chunks of
    128. Per superchunk (<=4 chunks): one natural dma_gather of xl rows
    (bf16), per-block gather of xr rows; s = xl+xr; leaky-relu on DVE;
    PE transposes -> att dot on TensorE -> exp on ScalarE -> per-edge weights;
    onehot (iota is_equal dst-slot, batched per superchunk) as matmul lhsT
    aggregates w*xl and the softmax denominators into PSUM across the block.
  - Segment softmax uses plain exp (scores are O(1); verified safe).
  - Epilogue: divide by denominators, bias, relu (L1) / LayerNorm (L2).
All compute in bf16 with fp32 PSUM accumulation; LayerNorm in fp32.
"""
import sys
import time

sys.path.insert(0, "/opt/trn_rl_repo")

import numpy as np
import ml_dtypes

import concourse.bass as bass
import concourse.bacc as bacc
import concourse.mybir as mybir
import concourse.tile as tile

bf16 = ml_dtypes.bfloat16
F32 = mybir.dt.float32
BF = mybir.dt.bfloat16
I16 = mybir.dt.int16

NCORES = 8
SLOPE = 0.2
EPS = 1e-5
SCMAX = 4  # chunks (of 128 edges) per superchunk


# ----------------------------------------------------------------- host prep

def _wrap_idxs(idx):
    """Flat int array -> [128, ceil(n/16)] int16 SWDGE layout (idx i at
    partition i%16, col i//16, replicated across the 8 groups of 16)."""
    idx = np.asarray(idx)
    n = len(idx)
    cols = (n + 15) // 16
    pad = np.zeros(cols * 16, np.int16)
    pad[:n] = idx.astype(np.int16)
    out = np.zeros((128, cols), np.int16)
    out[:16] = pad.reshape(cols, 16).T
    for g in range(1, 8):
        out[g * 16:(g + 1) * 16] = out[:16]
    return out


def _balance(dst, N, nbins):
    """Greedy assignment of nodes to nbins bins (<=128 nodes each),
    balancing total in-degree. Returns (node->bin, node->slot)."""
    import heapq
    deg = np.bincount(dst, minlength=N)
    order = np.argsort(-deg, kind="stable")
    nbin = np.zeros(N, np.int32)
    nslot = np.zeros(N, np.int32)
    heap = [(0, 0, b) for b in range(nbins)]
    heapq.heapify(heap)
    for i in order:
        c, n, b = heapq.heappop(heap)
        nbin[i] = b
        nslot[i] = n
        if n + 1 < 128:
            heapq.heappush(heap, (int(c + deg[i]), n + 1, b))
    return nbin, nslot


class _Prep:
    """All host-side preprocessing derived from edge_index + shapes."""

    def __init__(self, N, E, F_IN, HID, OUT, H, edge_index):
        self.N, self.F_IN, self.HID, self.OUT, self.H = N, F_IN, HID, OUT, H
        ei = np.asarray(edge_index)
        src = np.concatenate([ei[0], np.arange(N, dtype=np.int64)]).astype(np.int64)
        dst = np.concatenate([ei[1], np.arange(N, dtype=np.int64)]).astype(np.int64)
        self.NBLK = ((N + NCORES - 1) // NCORES + 127) // 128
        nbins = NCORES * self.NBLK
        nbin, nslot = _balance(dst, N, nbins)
        self.nbin, self.nslot = nbin, nslot
        core_of = nbin // self.NBLK
        blk_of = nbin % self.NBLK
        # per-bin edge lists sorted by dst slot
        ecore = core_of[dst]
        eblk = blk_of[dst]
        eslot = nslot[dst]
        bin_of_edge = nbin[dst]
        order = np.lexsort((eslot, bin_of_edge))
        src_s, bin_s, slot_s = src[order], bin_of_edge[order], eslot[order]
        counts = np.bincount(bin_s, minlength=nbins)
        self.CBLK = int(max(1, -(-counts.max() // 128)))
        S = self.CBLK * 128  # slots per bin
        # padded per-bin arrays
        self.esrc = np.zeros((nbins, S), np.int64)       # source node (orig id)
        self.eslot = np.full((nbins, S), -1.0, np.float32)  # dst slot or -1
        starts = np.concatenate([[0], np.cumsum(counts)])
        for b in range(nbins):
            n = counts[b]
            self.esrc[b, :n] = src_s[starts[b]:starts[b] + n]
            self.eslot[b, :n] = slot_s[starts[b]:starts[b] + n]
        # xr row (within the core's xr table): blk*128 + dst slot; pad -> 0
        dslot = np.where(self.eslot >= 0, self.eslot, 0).astype(np.int64)
        self.exr = np.zeros((nbins, S), np.int64)
        for b in range(nbins):
            self.exr[b] = (b % self.NBLK) * 128 + dslot[b]
        # layer-2 xl row for source j (XL2S layout [group][core][blk%2][slot]
        # where groups pair two blocks per AllGather of h)
        self.l2row = ((blk_of // 2) * (256 * NCORES) + core_of * 256
                      + (blk_of % 2) * 128 + nslot)
        # superchunk structure
        scs = []
        c = self.CBLK
        while c > 0:
            scs.append(min(SCMAX, c))
            c -= min(SCMAX, c)
        self.SCS = scs
        # per-core index/seg tables, wrapped PER SUPERCHUNK so that device
        # slices are contiguous. Layer 1 uses a combined [src | N+xr] list.
        self.idxl = [[None, None] for _ in range(NCORES)]
        self.idxr = [None] * NCORES
        self.seg = [None] * NCORES
        for core in range(NCORES):
            l1, l2, xr, sg = [], [], [], []
            for b in range(self.NBLK):
                g = core * self.NBLK + b
                ch = 0
                for C in self.SCS:
                    sl = slice(ch * 128, (ch + C) * 128)
                    l1.append(_wrap_idxs(np.concatenate(
                        [self.esrc[g][sl], N + self.exr[g][sl]])))
                    l2.append(_wrap_idxs(self.l2row[self.esrc[g][sl]]))
                    xr.append(_wrap_idxs(self.exr[g][sl]))
                    ch += C
                sg.append(self.eslot[g].reshape(self.CBLK, 128).T)
            self.idxl[core] = [np.concatenate(l1, 1), np.concatenate(l2, 1)]
            self.idxr[core] = np.concatenate(xr, 1)
            self.seg[core] = np.concatenate(sg, 1).astype(bf16)  # [128, NBLK*CBLK]
        self.blk_cols1 = self.idxl[0][0].shape[1] // self.NBLK    # combined l1
        self.blk_cols2 = self.idxl[0][1].shape[1] // self.NBLK
        self.xr_cols = self.idxr[0].shape[1] // self.NBLK


# --------------------------------------------------------------- device build

def _build_nc(p):
    N, F_IN, HID, OUT, H = p.N, p.F_IN, p.HID, p.OUT, p.H
    NBLK, CBLK, SCS = p.NBLK, p.CBLK, p.SCS
    XPAD = ((N + 127) // 128) * 128
    NOWN = NBLK * 128
    KIN = F_IN // 128
    K1 = HID // 128   # k-chunks of layer-2 dense input
    KF = {1: HID // 128, 2: OUT // 128}
    NROW2 = NBLK * 128 * NCORES

    nc = bacc.Bacc("TRN2", target_bir_lowering=False, debug=False,
                   num_devices=NCORES)
    # ---- external inputs
    xT = nc.dram_tensor("xT", [F_IN, XPAD], BF, kind="ExternalInput")
    xownT = nc.dram_tensor("xownT", [F_IN, NOWN], BF, kind="ExternalInput")
    WlT1 = nc.dram_tensor("WlT1", [F_IN, HID], BF, kind="ExternalInput")
    WrT1 = nc.dram_tensor("WrT1", [F_IN, HID], BF, kind="ExternalInput")
    WlT2 = nc.dram_tensor("WlT2", [HID, OUT], BF, kind="ExternalInput")
    WrT2 = nc.dram_tensor("WrT2", [HID, OUT], BF, kind="ExternalInput")
    att1 = nc.dram_tensor("att1", [HID, H], BF, kind="ExternalInput")
    att2 = nc.dram_tensor("att2", [OUT, H], BF, kind="ExternalInput")
    b1rep = nc.dram_tensor("b1rep", [128, HID], F32, kind="ExternalInput")
    b2rep = nc.dram_tensor("b2rep", [128, OUT], F32, kind="ExternalInput")
    gam = nc.dram_tensor("gam", [128, OUT], F32, kind="ExternalInput")
    bet = nc.dram_tensor("bet", [128, OUT], F32, kind="ExternalInput")
    iota = nc.dram_tensor("iota", [128, 128], BF, kind="ExternalInput")
    ident = nc.dram_tensor("ident", [128, 128], BF, kind="ExternalInput")
    identf = nc.dram_tensor("identf", [128, 128], F32, kind="ExternalInput")
    blr1 = nc.dram_tensor("blr1", [1, HID], BF, kind="ExternalInput")
    brr1 = nc.dram_tensor("brr1", [1, HID], BF, kind="ExternalInput")
    blr2 = nc.dram_tensor("blr2", [1, OUT], BF, kind="ExternalInput")
    brr2 = nc.dram_tensor("brr2", [1, OUT], BF, kind="ExternalInput")
    idxl1 = nc.dram_tensor("idxl1", list(p.idxl[0][0].shape), I16, kind="ExternalInput")
    idxl2 = nc.dram_tensor("idxl2", list(p.idxl[0][1].shape), I16, kind="ExternalInput")
    idxr = nc.dram_tensor("idxr", list(p.idxr[0].shape), I16, kind="ExternalInput")
    seg = nc.dram_tensor("seg", list(p.seg[0].shape), BF, kind="ExternalInput")
    # ---- outputs
    out_o = nc.dram_tensor("out_o", [NOWN, OUT], F32, kind="ExternalOutput")
    # ---- internal dram
    NG = (NBLK + 1) // 2  # block pairs per h-AllGather
    XLR1 = nc.dram_tensor("XLR1", [N + NOWN, HID], BF)
    xr2tab = nc.dram_tensor("xr2tab", [NOWN, OUT], BF)
    hownT = nc.dram_tensor("hownT", [NG, HID, 256], BF)
    HST = nc.dram_tensor("HST", [NG * NCORES * HID, 256], BF,
                         addr_space="Shared")
    XL2S = nc.dram_tensor("XL2S", [NG * NCORES * 256, OUT], BF)
    warm_in = nc.dram_tensor("warm_in", [1, 64], F32)
    warm_out = nc.dram_tensor("warm_out", [1, 64], F32, addr_space="Shared")

    with tile.TileContext(nc) as tc:
        with (
            tc.tile_pool(name="cons", bufs=1) as cons,
            tc.tile_pool(name="ps_dense", bufs=1, space="PSUM") as psd_pool,
        ):
            # comm-path warmup: tiny AllReduce issued first so the
            # one-time collective initialization overlaps dense/edge compute
            wt = cons.tile([1, 64], F32, tag="warm")
            nc.vector.memset(wt[:], 1.0)
            nc.sync.dma_start(warm_in[:], wt[:])
            nc.gpsimd.collective_compute(
                "AllReduce", mybir.AluOpType.add,
                replica_groups=[list(range(NCORES))],
                ins=[warm_in[:].opt()], outs=[warm_out[:].opt()])
            # ---------------- constants
            wl1 = cons.tile([128, KIN, HID], BF)
            wr1 = cons.tile([128, KIN, HID], BF)
            wl2 = cons.tile([128, K1, OUT], BF)
            wr2 = cons.tile([128, K1, OUT], BF)
            for k in range(KIN):
                nc.sync.dma_start(wl1[:, k, :], WlT1[k * 128:(k + 1) * 128, :])
                nc.sync.dma_start(wr1[:, k, :], WrT1[k * 128:(k + 1) * 128, :])
            for k in range(K1):
                nc.sync.dma_start(wl2[:, k, :], WlT2[k * 128:(k + 1) * 128, :])
                nc.sync.dma_start(wr2[:, k, :], WrT2[k * 128:(k + 1) * 128, :])
            att_t = {}
            for lay, (attd, Fo) in {1: (att1, HID), 2: (att2, OUT)}.items():
                a = cons.tile([128, Fo // 128, H], BF, tag=f"att{lay}")
                for k in range(Fo // 128):
                    nc.sync.dma_start(a[:, k, :], attd[k * 128:(k + 1) * 128, :])
                att_t[lay] = a
            b1rep_t = cons.tile([128, HID], F32)
            b2rep_t = cons.tile([128, OUT], F32)
            gam_t = cons.tile([128, OUT], F32)
            bet_t = cons.tile([128, OUT], F32)
            iota_t = cons.tile([128, 128], BF)
            id_t = cons.tile([128, 128], BF)
            idf_t = cons.tile([128, 128], F32)
            for t, d in [(b1rep_t, b1rep), (b2rep_t, b2rep), (gam_t, gam),
                         (bet_t, bet), (iota_t, iota), (id_t, ident),
                         (idf_t, identf)]:
                nc.sync.dma_start(t[:], d[:])
            ones_t = cons.tile([1, 128], BF)
            nc.vector.memset(ones_t[:], 1.0)
            eps_t = cons.tile([128, 1], F32)
            nc.vector.memset(eps_t[:], EPS)
            blr1_t = cons.tile([1, HID], BF, tag="blr1")
            brr1_t = cons.tile([1, HID], BF, tag="brr1")
            blr2_t = cons.tile([1, OUT], BF, tag="blr2")
            brr2_t = cons.tile([1, OUT], BF, tag="brr2")
            for t, d in [(blr1_t, blr1), (brr1_t, brr1), (blr2_t, blr2),
                         (brr2_t, brr2)]:
                nc.sync.dma_start(t[:], d[:])
            idxl1_t = cons.tile(list(p.idxl[0][0].shape), I16)
            idxl2_t = cons.tile(list(p.idxl[0][1].shape), I16)
            idxr_t = cons.tile(list(p.idxr[0].shape), I16)
            seg_t = cons.tile(list(p.seg[0].shape), BF)
            nc.sync.dma_start(idxl1_t[:], idxl1[:])
            nc.sync.dma_start(idxl2_t[:], idxl2[:])
            nc.sync.dma_start(idxr_t[:], idxr[:])
            nc.sync.dma_start(seg_t[:], seg[:])

            # ---------------- dense layer 1 (replicated)
            with tc.tile_pool(name="d1", bufs=1) as d1p, \
                 tc.tile_pool(name="d1w", bufs=3) as d1w:
                xT_t = d1p.tile([128, KIN, XPAD], BF)
                for k in range(KIN):
                    nc.sync.dma_start(xT_t[:, k, :], xT[k * 128:(k + 1) * 128, :])
                xoT_t = d1p.tile([128, KIN, NOWN], BF)
                for k in range(KIN):
                    nc.sync.dma_start(xoT_t[:, k, :], xownT[k * 128:(k + 1) * 128, :])

                def dense(lhsT, w_t, kk, Fo, rows, dst, dst_off, tag, brow):
                    ps = psd_pool.tile([128, Fo], F32, tag="dense")
                    for k in range(kk):
                        nc.tensor.matmul(ps[:], lhsT[:, k, :], w_t[:, k, :],
                                         start=(k == 0),
                                         stop=(k == kk - 1 and brow is None))
                    if brow is not None:
                        nc.tensor.matmul(ps[:], ones_t[:], brow[:],
                                         start=False, stop=True)
                    o = d1w.tile([128, Fo], BF, tag=tag)
                    nc.scalar.copy(o[:], ps[:])
                    nc.sync.dma_start(dst[dst_off:dst_off + rows, :], o[:rows, :])

                bl1b = blr1_t if p.use_bias else None
                br1b = brr1_t if p.use_bias else None
                for ch in range(XPAD // 128):
                    rows = min(128, N - ch * 128)
                    if rows <= 0:
                        break
                    lhsT = xT_t[:, :, ch * 128:ch * 128 + 128]
                    dense(lhsT, wl1, KIN, HID, rows, XLR1, ch * 128, "d1o", bl1b)
                for b in range(NBLK):
                    lhsT = xoT_t[:, :, b * 128:(b + 1) * 128]
                    dense(lhsT, wr1, KIN, HID, 128, XLR1, N + b * 128, "d1o", br1b)

            # ---------------- edge phases (layer-scoped pools)
            def emit_edge_block(pools, lay, b, Fo):
                (g_pool, ew_pool, eo_pool, pst_pool, psv_pool, psdn_pool,
                 pse_pool, pss_pool) = pools
                kf = Fo // 128
                psv = psv_pool.tile([128, Fo], F32, tag="aggv")
                psd = psdn_pool.tile([4, 128], F32, tag="aggd")
                ch = 0
                off1 = 0  # running col offset of combined l1 idx within block
                off2 = 0
                for C in SCS:
                    if lay == 1:
                        cols0 = b * p.blk_cols1 + off1
                        g = g_pool.tile([128, 2 * SCMAX, Fo], BF, tag="g1")
                        nc.gpsimd.dma_gather(
                            g[:, :2 * C, :], XLR1[:],
                            idxl1_t[:, cols0:cols0 + 2 * C * 8],
                            2 * C * 128, 2 * C * 128, Fo)
                        xlg = g[:, :C, :]
                        xrg = g[:, C:2 * C, :]
                    else:
                        cols0 = b * p.blk_cols2 + off2
                        colsr = b * p.xr_cols + off2
                        gl = g_pool.tile([128, SCMAX, Fo], BF, tag="gl2")
                        nc.gpsimd.dma_gather(
                            gl[:, :C, :], XL2S[:],
                            idxl2_t[:, cols0:cols0 + C * 8],
                            C * 128, C * 128, Fo)
                        gr = g_pool.tile([128, SCMAX, Fo], BF, tag="gr2")
                        nc.gpsimd.dma_gather(
                            gr[:, :C, :], xr2tab[:],
                            idxr_t[:, colsr:colsr + C * 8],
                            C * 128, C * 128, Fo)
                        xlg = gl[:, :C, :]
                        xrg = gr[:, :C, :]
                    off1 += 2 * C * 8
                    off2 += C * 8
                    s = ew_pool.tile([128, SCMAX, Fo], BF, tag="s")
                    nc.vector.tensor_tensor(s[:, :C, :], xlg, xrg,
                                            op=mybir.AluOpType.add)
                    t = ew_pool.tile([128, SCMAX, Fo], BF, tag="t")
                    nc.vector.scalar_tensor_tensor(
                        t[:, :C, :], s[:, :C, :], SLOPE, s[:, :C, :],
                        op0=mybir.AluOpType.mult, op1=mybir.AluOpType.max)
                    # PE transposes of t chunks into PSUM (bank groups of 8)
                    ntp = C * kf
                    tT_ps = pst_pool.tile([128, SCMAX * kf * 128], BF, tag="tT")
                    for i in range(ntp):
                        c, k = divmod(i, kf)
                        nc.tensor.matmul(
                            tT_ps[:, i * 128:(i + 1) * 128],
                            t[:, c, k * 128:(k + 1) * 128], id_t[:],
                            is_transpose=True,
                            start=(i % 8 == 0),
                            stop=(i % 8 == 7) or (i == ntp - 1))
                    tT = ew_pool.tile([128, SCMAX * kf * 128], BF, tag="tTs")
                    nc.scalar.copy(tT[:, :ntp * 128], tT_ps[:, :ntp * 128])
                    # dot with att: e[h, c*128+e'] accumulated over kf
                    e_ps = pse_pool.tile([4, SCMAX * 128], F32, tag="e")
                    tT3 = tT[:, :ntp * 128].rearrange(
                        "p (c k e) -> p c k e", c=C, k=kf)
                    for k in range(kf):
                        nc.tensor.matmul(
                            e_ps[:, :C * 128].rearrange("h (c e) -> h c e", c=C),
                            att_t[lay][:, k, :], tT3[:, :, k, :],
                            start=(k == 0), stop=(k == kf - 1))
                    wT = ew_pool.tile([4, SCMAX * 128], BF, tag="wT")
                    nc.scalar.activation(wT[:, :C * 128], e_ps[:, :C * 128],
                                         mybir.ActivationFunctionType.Exp)
                    wn_ps = pss_pool.tile([128, SCMAX * 4], BF, tag="small")
                    for c in range(C):
                        nc.tensor.matmul(
                            wn_ps[:, c * 4:(c + 1) * 4],
                            wT[:, c * 128:(c + 1) * 128], id_t[:4, :4],
                            is_transpose=True,
                            start=(c == 0), stop=(c == C - 1))
                    wn = ew_pool.tile([128, SCMAX, 4], BF, tag="wns")
                    nc.vector.tensor_copy(
                        wn[:, :C, :].rearrange("p c h -> p (c h)"),
                        wn_ps[:, :C * 4])
                    oh = ew_pool.tile([128, SCMAX, 128], BF, tag="oh")
                    nc.vector.tensor_tensor(
                        out=oh[:, :C, :],
                        in0=iota_t[:].unsqueeze(1).to_broadcast((128, C, 128)),
                        in1=seg_t[:, b * CBLK + ch:b * CBLK + ch + C]
                            .unsqueeze(2).to_broadcast((128, C, 128)),
                        op=mybir.AluOpType.is_equal)
                    v = ew_pool.tile([128, SCMAX, Fo], BF, tag="v")
                    nc.vector.tensor_tensor(
                        out=v[:, :C, :].rearrange("p c (h d) -> p c h d", h=H),
                        in0=xlg.rearrange("p c (h d) -> p c h d", h=H),
                        in1=wn[:, :C, :].unsqueeze(3).to_broadcast(
                            (128, C, H, Fo // H)),
                        op=mybir.AluOpType.mult)
                    for c in range(C):
                        cc = ch + c
                        nc.tensor.matmul(psv[:], oh[:, c, :], v[:, c, :],
                                         start=(cc == 0), stop=(cc == CBLK - 1))
                        nc.tensor.matmul(psd[:], wn[:, c, :], oh[:, c, :],
                                         start=(cc == 0), stop=(cc == CBLK - 1))
                    ch += C
                # ---- epilogue: normalize by denominators
                denT = ew_pool.tile([4, 128], F32, tag="denT")
                nc.vector.tensor_scalar_add(denT[:], psd[:], 1e-30)
                rec = ew_pool.tile([4, 128], F32, tag="rec")
                nc.vector.reciprocal(rec[:], denT[:])
                rec_ps = pss_pool.tile([128, 4], F32, tag="small")
                nc.tensor.matmul(rec_ps[:], rec[:], idf_t[:4, :4],
                                 is_transpose=True)
                recn = ew_pool.tile([128, 4], F32, tag="recn")
                nc.vector.tensor_copy(recn[:], rec_ps[:])
                vn = eo_pool.tile([128, Fo], F32, tag="vn")
                nc.vector.tensor_tensor(
                    out=vn[:].rearrange("p (h d) -> p h d", h=H),
                    in0=psv[:].rearrange("p (h d) -> p h d", h=H),
                    in1=recn[:].unsqueeze(2).to_broadcast((128, H, Fo // H)),
                    op=mybir.AluOpType.mult)
                return vn

            with (
                tc.tile_pool(name="ps_v", bufs=2, space="PSUM") as psv_pool,
                tc.tile_pool(name="ps_d", bufs=1, space="PSUM") as psdn_pool,
                tc.tile_pool(name="ps_e", bufs=1, space="PSUM") as pse_pool,
                tc.tile_pool(name="ps_s", bufs=1, space="PSUM") as pss_pool,
            ):
                psum_pools = (psv_pool, psdn_pool, pse_pool, pss_pool)
                # ======== layer 1 blocks + dense layer 2 + allgather
                with (
                    tc.tile_pool(name="g1", bufs=4) as g_pool,
                    tc.tile_pool(name="ew1", bufs=4) as ew_pool,
                    tc.tile_pool(name="eo1", bufs=2) as eo_pool,
                    tc.tile_pool(name="ps_t1", bufs=2, space="PSUM") as pst_pool,
                ):
                    pools = (g_pool, ew_pool, eo_pool, pst_pool) + psum_pools

                    def dense_xl2_group(g):
                        """xl2 for group g's gathered h rows (all cores)."""
                        gsz = min(2, NBLK - g * 2)
                        for core in range(NCORES):
                            htt = eo_pool.tile([128, K1, 256], BF, tag="htt")
                            for k in range(K1):
                                r0 = (g * NCORES + core) * HID + k * 128
                                nc.sync.dma_start(
                                    htt[:, k, :gsz * 128],
                                    HST[r0:r0 + 128, :gsz * 128])
                            for j in range(gsz):
                                ps = psd_pool.tile([128, OUT], F32, tag="dense")
                                for k in range(K1):
                                    nc.tensor.matmul(
                                        ps[:], htt[:, k, j * 128:(j + 1) * 128],
                                        wl2[:, k, :], start=(k == 0),
                                        stop=(k == K1 - 1 and not p.use_bias))
                                if p.use_bias:
                                    nc.tensor.matmul(ps[:], ones_t[:], blr2_t[:],
                                                     start=False, stop=True)
                                o = eo_pool.tile([128, OUT], BF, tag="d2o")
                                nc.vector.tensor_copy(o[:], ps[:])
                                base = (g * NCORES + core) * 256 + j * 128
                                nc.sync.dma_start(XL2S[base:base + 128, :], o[:])

                    for b in range(NBLK):
                        vn = emit_edge_block(pools, 1, b, HID)
                        vb = eo_pool.tile([128, HID], F32, tag="vb1")
                        nc.vector.tensor_tensor(vb[:], vn[:], b1rep_t[:],
                                                op=mybir.AluOpType.add)
                        h = eo_pool.tile([128, HID], BF, tag="h1")
                        nc.scalar.activation(h[:], vb[:],
                                             mybir.ActivationFunctionType.Relu)
                        # transposed h for this block -> hownT + xr2 dense
                        hT_ps = pss_pool.tile([128, HID], BF, tag="small")
                        for k in range(K1):
                            nc.tensor.matmul(hT_ps[:, k * 128:(k + 1) * 128],
                                             h[:, k * 128:(k + 1) * 128], id_t[:],
                                             is_transpose=True,
                                             start=(k == 0), stop=(k == K1 - 1))
                        hT = eo_pool.tile([128, HID], BF, tag="hTs")
                        nc.scalar.copy(hT[:], hT_ps[:])
                        for k in range(K1):
                            nc.sync.dma_start(
                                hownT[b // 2, k * 128:(k + 1) * 128,
                                      (b % 2) * 128:(b % 2 + 1) * 128],
                                hT[:, k * 128:(k + 1) * 128])
                        ps = psd_pool.tile([128, OUT], F32, tag="dense")
                        for k in range(K1):
                            nc.tensor.matmul(ps[:], hT[:, k * 128:(k + 1) * 128],
                                             wr2[:, k, :], start=(k == 0),
                                             stop=(k == K1 - 1 and p.use_bias is False))
                        if p.use_bias:
                            nc.tensor.matmul(ps[:], ones_t[:], brr2_t[:],
                                             start=False, stop=True)
                        o = eo_pool.tile([128, OUT], BF, tag="d2o")
                        nc.scalar.copy(o[:], ps[:])
                        nc.sync.dma_start(xr2tab[b * 128:(b + 1) * 128, :], o[:])
                        # AllGather transposed h per pair of blocks
                        if b % 2 == 1 or b == NBLK - 1:
                            g = b // 2
                            nc.gpsimd.collective_compute(
                                "AllGather", mybir.AluOpType.bypass,
                                replica_groups=[list(range(NCORES))],
                                ins=[hownT[g].opt()],
                                outs=[HST[g * NCORES * HID:(g + 1) * NCORES * HID,
                                          :].opt()])
                        # overlap: xl2 dense for the PREVIOUS group
                        if b % 2 == 1 and b >= 3:
                            dense_xl2_group(b // 2 - 1)
                    dense_xl2_group(NG - 1)

                # ======== layer 2 blocks + layernorm
                with (
                    tc.tile_pool(name="g2", bufs=4) as g_pool,
                    tc.tile_pool(name="ew2", bufs=4) as ew_pool,
                    tc.tile_pool(name="eo2", bufs=2) as eo_pool,
                    tc.tile_pool(name="ps_t2", bufs=1, space="PSUM") as pst_pool,
                ):
                    pools = (g_pool, ew_pool, eo_pool, pst_pool) + psum_pools
                    for b in range(NBLK):
                        vn = emit_edge_block(pools, 2, b, OUT)
                        vb = eo_pool.tile([128, OUT], F32, tag="vb2")
                        nc.vector.tensor_tensor(vb[:], vn[:], b2rep_t[:],
                                                op=mybir.AluOpType.add)
                        # layernorm over OUT
                        tmp = eo_pool.tile([128, OUT], F32, tag="lntmp")
                        ssum = eo_pool.tile([128, 1], F32, tag="lnsum")
                        nc.scalar.activation(tmp[:], vb[:],
                                             mybir.ActivationFunctionType.Copy,
                                             accum_out=ssum[:])
                        negmu = eo_pool.tile([128, 1], F32, tag="lnmu")
                        nc.vector.tensor_scalar_mul(negmu[:], ssum[:], -1.0 / OUT)
                        xm = eo_pool.tile([128, OUT], F32, tag="lnxm")
                        nc.scalar.activation(xm[:], vb[:],
                                             mybir.ActivationFunctionType.Identity,
                                             bias=negmu[:])
                        sq = eo_pool.tile([128, OUT], F32, tag="lnsq")
                        ssq = eo_pool.tile([128, 1], F32, tag="lnssq")
                        nc.scalar.activation(sq[:], xm[:],
                                             mybir.ActivationFunctionType.Square,
                                             accum_out=ssq[:])
                        sd = eo_pool.tile([128, 1], F32, tag="lnsd")
                        nc.scalar.activation(sd[:], ssq[:],
                                             mybir.ActivationFunctionType.Sqrt,
                                             scale=1.0 / OUT, bias=eps_t[:])
                        rstd = eo_pool.tile([128, 1], F32, tag="lnrstd")
                        nc.vector.reciprocal(rstd[:], sd[:])
                        og = eo_pool.tile([128, OUT], F32, tag="lnog")
                        nc.vector.scalar_tensor_tensor(
                            og[:], xm[:], rstd[:], gam_t[:],
                            op0=mybir.AluOpType.mult, op1=mybir.AluOpType.mult)
                        ob = eo_pool.tile([128, OUT], F32, tag="lnob")
                        nc.vector.tensor_tensor(ob[:], og[:], bet_t[:],
                                                op=mybir.AluOpType.add)
                        nc.sync.dma_start(out_o[b * 128:(b + 1) * 128, :], ob[:])

    nc.compile()
    return nc


# --------------------------------------------------------------- input maps

def _make_in_maps(p, inputs):
    N, F_IN, HID, OUT, H = p.N, p.F_IN, p.HID, p.OUT, p.H
    XPAD = ((N + 127) // 128) * 128
    NOWN = p.NBLK * 128
    x = np.asarray(inputs["x"], np.float32)
    xpad = np.zeros((XPAD, F_IN), np.float32)
    xpad[:N] = x
    xT = np.ascontiguousarray(xpad.T).astype(bf16)

    def attfull(att, Fo):
        H_, d = att.shape
        a = np.zeros((Fo, H_), np.float32)
        for h in range(H_):
            a[h * d:(h + 1) * d, h] = att[h]
        return a.astype(bf16)

    Wl1 = np.asarray(inputs["Wl1"], np.float32)
    Wr1 = np.asarray(inputs["Wr1"], np.float32)
    Wl2 = np.asarray(inputs["Wl2"], np.float32)
    Wr2 = np.asarray(inputs["Wr2"], np.float32)
    common = dict(
        xT=xT,
        WlT1=np.ascontiguousarray(Wl1.T).astype(bf16),
        WrT1=np.ascontiguousarray(Wr1.T).astype(bf16),
        WlT2=np.ascontiguousarray(Wl2.T).astype(bf16),
        WrT2=np.ascontiguousarray(Wr2.T).astype(bf16),
        att1=attfull(np.asarray(inputs["att1"], np.float32), HID),
        att2=attfull(np.asarray(inputs["att2"], np.float32), OUT),
        b1rep=np.broadcast_to(
            np.asarray(inputs["bias1"], np.float32), (128, HID)).copy(),
        b2rep=np.broadcast_to(
            np.asarray(inputs["bias2"], np.float32), (128, OUT)).copy(),
        gam=np.broadcast_to(
            np.asarray(inputs["gamma"], np.float32), (128, OUT)).copy(),
        bet=np.broadcast_to(
            np.asarray(inputs["beta"], np.float32), (128, OUT)).copy(),
        iota=np.broadcast_to(
            np.arange(128, dtype=np.float32), (128, 128)).astype(bf16),
        ident=np.eye(128, dtype=np.float32).astype(bf16),
        identf=np.eye(128, dtype=np.float32),
    )
    common["blr1"] = np.asarray(inputs["bl1"], np.float32).reshape(1, HID).astype(bf16)
    common["brr1"] = np.asarray(inputs["br1"], np.float32).reshape(1, HID).astype(bf16)
    common["blr2"] = np.asarray(inputs["bl2"], np.float32).reshape(1, OUT).astype(bf16)
    common["brr2"] = np.asarray(inputs["br2"], np.float32).reshape(1, OUT).astype(bf16)
    in_maps = []
    for c in range(NCORES):
        m = dict(common)
        # per-core x_own columns (slot order)
        xo = np.zeros((NOWN, F_IN), np.float32)
        for b in range(p.NBLK):
            g = c * p.NBLK + b
            nodes = np.nonzero(p.nbin == g)[0]
            xo[b * 128 + p.nslot[nodes]] = x[nodes]
        m["xownT"] = np.ascontiguousarray(xo.T).astype(bf16)
        m["idxl1"] = p.idxl[c][0]
        m["idxl2"] = p.idxl[c][1]
        m["idxr"] = p.idxr[c]
        m["seg"] = p.seg[c]
        in_maps.append(m)
    return in_maps


# ----------------------------------------------------------------- runner

class _Runner:
    def __init__(self, inputs):
        ei = np.asarray(inputs["edge_index"])
        N, F_IN = np.asarray(inputs["x"]).shape
        HID = np.asarray(inputs["Wl1"]).shape[0]
        OUT = np.asarray(inputs["Wl2"]).shape[0]
        H = np.asarray(inputs["att1"]).shape[0]
        self.eihash = hash(ei.tobytes())
        self.p = _Prep(N, ei.shape[1], F_IN, HID, OUT, H, ei)
        self.p.use_bias = any(
            np.abs(np.asarray(inputs[k])).max() > 0
            for k in ("bl1", "br1", "bl2", "br2"))
        self.nc = _build_nc(self.p)
        self.jit_fn = None

    def _prep_jit(self):
        """Build the shard_map jit once (mirrors bass2jax.run_bass_via_pjrt)."""
        import jax
        from jax.sharding import Mesh, PartitionSpec
        from jax.experimental.shard_map import shard_map
        from concourse import bass2jax
        from concourse.bass2jax import _bass_exec_p, partition_id_tensor
        nc = self.nc
        bass2jax.install_neuronx_cc_hook()
        pname = nc.partition_id_tensor.name if nc.partition_id_tensor else None
        in_names, out_names, out_avals, zero_outs = [], [], [], []
        for alloc in nc.m.functions[0].allocations:
            if not isinstance(alloc, mybir.MemoryLocationSet):
                continue
            name = alloc.memorylocations[0].name
            if alloc.kind == "ExternalInput":
                if name != pname:
                    in_names.append(name)
            elif alloc.kind == "ExternalOutput":
                out_names.append(name)
                shape = tuple(alloc.tensor_shape)
                dtype = mybir.dt.np(alloc.dtype)
                out_avals.append(jax.core.ShapedArray(shape, dtype))
                zero_outs.append(np.zeros(shape, dtype))
        n_params = len(in_names)
        all_names = in_names + out_names
        if pname is not None:
            all_names = all_names + [pname]

        def _body(*args):
            operands = list(args)
            if pname is not None:
                operands.append(partition_id_tensor())
            outs = _bass_exec_p.bind(
                *operands, out_avals=tuple(out_avals), in_names=tuple(all_names),
                out_names=tuple(out_names), lowering_input_output_aliases=(),
                sim_require_finite=True, sim_require_nnan=True, nc=nc)
            return tuple(outs)

        devices = jax.devices()[:NCORES]
        mesh = Mesh(np.asarray(devices), ("core",))
        n_outs = len(out_names)
        self.jit_fn = jax.jit(
            shard_map(_body, mesh=mesh,
                      in_specs=(PartitionSpec("core"),) * (n_params + n_outs),
                      out_specs=(PartitionSpec("core"),) * n_outs,
                      check_rep=False),
            keep_unused=True)
        self.in_names = in_names
        self.out_names = out_names
        self.out_avals = out_avals
        self.zero_outs = zero_outs
        self.mesh = mesh

    def device_args(self, inputs):
        in_maps = _make_in_maps(self.p, inputs)
        concat_in = [np.concatenate([in_maps[c][n] for c in range(NCORES)], 0)
                     for n in self.in_names]
        concat_zero = [np.zeros((NCORES * z.shape[0], *z.shape[1:]), z.dtype)
                       for z in self.zero_outs]
        return concat_in + concat_zero

    def run(self, inputs):
        if self.jit_fn is None:
            self._prep_jit()
        args = self.device_args(inputs)
        out_arrs = self.jit_fn(*args)
        res = [
            {n: np.asarray(out_arrs[i]).reshape(
                NCORES, *self.out_avals[i].shape)[c]
             for i, n in enumerate(self.out_names)}
            for c in range(NCORES)
        ]
        return self.assemble(res)

    def assemble(self, res):
        p = self.p
        out = np.zeros((p.N, p.OUT), np.float32)
        for c in range(NCORES):
            o = np.asarray(res[c]["out_o"], np.float32)
            for b in range(p.NBLK):
                g = c * p.NBLK + b
                nodes = np.nonzero(p.nbin == g)[0]
                out[nodes] = o[b * 128 + p.nslot[nodes]]
        return out

    def timed_loop(self, inputs, r1=4, r2=40, reps=2):
        """Async-pipelined dispatch timing; difference two batch sizes to
        cancel fixed per-batch overhead."""
        import jax
        from jax.sharding import NamedSharding, PartitionSpec
        if self.jit_fn is None:
            self._prep_jit()
        args = self.device_args(inputs)
        sh = NamedSharding(self.mesh, PartitionSpec("core"))
        dargs = [jax.device_put(a, sh) for a in args]
        jax.block_until_ready(dargs)
        out = self.jit_fn(*dargs)
        jax.block_until_ready(out)

        def batch(R):
            ts = []
            for _ in range(reps):
                t0 = time.perf_counter()
                outs = [self.jit_fn(*dargs) for _ in range(R)]
                jax.block_until_ready(outs)
                ts.append(time.perf_counter() - t0)
            return min(ts)

        t1, t2 = batch(r1), batch(r2)
        return (t2 - t1) / (r2 - r1) * 1e9

    def timed(self, inputs, reps=5):
        import jax
        from jax.sharding import NamedSharding, PartitionSpec
        if self.jit_fn is None:
            self._prep_jit()
        args = self.device_args(inputs)
        sh = NamedSharding(self.mesh, PartitionSpec("core"))
        dargs = [jax.device_put(a, sh) for a in args]
        jax.block_until_ready(dargs)
        out = self.jit_fn(*dargs)
        jax.block_until_ready(out)
        times = []
        for _ in range(reps):
            t0 = time.perf_counter()
            out = self.jit_fn(*dargs)
            jax.block_until_ready(out)
            times.append(time.perf_counter() - t0)
        return min(times) * 1e9


_CACHE = {}


def kernel(**inputs):
    ei = np.asarray(inputs["edge_index"])
    key = hash(ei.tobytes())
    if key not in _CACHE:
        _CACHE.clear()
        _CACHE[key] = _Runner(inputs)
    r = _CACHE[key]
    try:
        return r.run(inputs)
    except Exception:
        # fallback: the plain concourse SPMD runner
        from concourse.bass_utils import run_bass_kernel_spmd
        in_maps = _make_in_maps(r.p, inputs)
        res = run_bass_kernel_spmd(r.nc, in_maps, list(range(NCORES)))
        return r.assemble(res.results)



# revision 15
# speedup vs baseline: 2.5842x; 2.5842x over previous
"""Trainium2 Bass kernel for a 2-layer GATv2 encoder + LayerNorm (ASTGATEncoder).

Strategy (8 NeuronCores, SPMD single NEFF):
  - Nodes are greedily balanced into (core, block) bins of <=128 dst nodes,
    equalizing in-edge counts. Each core owns 1/8 of the nodes (10 blocks).
  - Dense transforms: layer 1 replicated (xl1 for all nodes per core, xr1 for
    own nodes); layer 2: transposed h AllGathered per pair of blocks (half the
    bytes of xl2), xl2 for all gathered rows then computed locally, pipelined
    against the layer-1 edge loop.
  - Edge phase per block: edges sorted by dst slot, padded to CBLK chunks of
    128. Per superchunk (<=4 chunks): one natural dma_gather of xl rows
    (bf16), per-block gather of xr rows; s = xl+xr; leaky-relu on DVE;
    PE transposes -> att dot on TensorE -> exp on ScalarE -> per-edge weights;
    onehot (iota is_equal dst-slot, batched per superchunk) as matmul lhsT
    aggregates w*xl and the softmax denominators into PSUM across the block.
  - Segment softmax uses plain exp (scores are O(1); verified safe).
  - Epilogue: divide by denominators, bias, relu (L1) / LayerNorm (L2).
All compute in bf16 with fp32 PSUM accumulation; LayerNorm in fp32.
"""
import sys
import time

sys.path.insert(0, "/opt/trn_rl_repo")

import numpy as np
import ml_dtypes

import concourse.bass as bass
import concourse.bacc as bacc
import concourse.mybir as mybir
import concourse.tile as tile

bf16 = ml_dtypes.bfloat16
F32 = mybir.dt.float32
BF = mybir.dt.bfloat16
I16 = mybir.dt.int16

NCORES = 8
SLOPE = 0.2
EPS = 1e-5
SCMAX = 4  # chunks (of 128 edges) per superchunk


# ----------------------------------------------------------------- host prep

def _wrap_idxs(idx):
    """Flat int array -> [128, ceil(n/16)] int16 SWDGE layout (idx i at
    partition i%16, col i//16, replicated across the 8 groups of 16)."""
    idx = np.asarray(idx)
    n = len(idx)
    cols = (n + 15) // 16
    pad = np.zeros(cols * 16, np.int16)
    pad[:n] = idx.astype(np.int16)
    out = np.zeros((128, cols), np.int16)
    out[:16] = pad.reshape(cols, 16).T
    for g in range(1, 8):
        out[g * 16:(g + 1) * 16] = out[:16]
    return out


def _balance(dst, N, nbins):
    """Greedy assignment of nodes to nbins bins (<=128 nodes each),
    balancing total in-degree. Returns (node->bin, node->slot)."""
    import heapq
    deg = np.bincount(dst, minlength=N)
    order = np.argsort(-deg, kind="stable")
    nbin = np.zeros(N, np.int32)
    nslot = np.zeros(N, np.int32)
    heap = [(0, 0, b) for b in range(nbins)]
    heapq.heapify(heap)
    for i in order:
        c, n, b = heapq.heappop(heap)
        nbin[i] = b
        nslot[i] = n
        if n + 1 < 128:
            heapq.heappush(heap, (int(c + deg[i]), n + 1, b))
    return nbin, nslot


class _Prep:
    """All host-side preprocessing derived from edge_index + shapes."""

    def __init__(self, N, E, F_IN, HID, OUT, H, edge_index):
        self.N, self.F_IN, self.HID, self.OUT, self.H = N, F_IN, HID, OUT, H
        ei = np.asarray(edge_index)
        src = np.concatenate([ei[0], np.arange(N, dtype=np.int64)]).astype(np.int64)
        dst = np.concatenate([ei[1], np.arange(N, dtype=np.int64)]).astype(np.int64)
        self.NBLK = ((N + NCORES - 1) // NCORES + 127) // 128
        nbins = NCORES * self.NBLK
        nbin, nslot = _balance(dst, N, nbins)
        self.nbin, self.nslot = nbin, nslot
        core_of = nbin // self.NBLK
        blk_of = nbin % self.NBLK
        # per-bin edge lists sorted by dst slot
        ecore = core_of[dst]
        eblk = blk_of[dst]
        eslot = nslot[dst]
        bin_of_edge = nbin[dst]
        order = np.lexsort((eslot, bin_of_edge))
        src_s, bin_s, slot_s = src[order], bin_of_edge[order], eslot[order]
        counts = np.bincount(bin_s, minlength=nbins)
        self.CBLK = int(max(1, -(-counts.max() // 128)))
        S = self.CBLK * 128  # slots per bin
        # padded per-bin arrays
        self.esrc = np.zeros((nbins, S), np.int64)       # source node (orig id)
        self.eslot = np.full((nbins, S), -1.0, np.float32)  # dst slot or -1
        starts = np.concatenate([[0], np.cumsum(counts)])
        for b in range(nbins):
            n = counts[b]
            self.esrc[b, :n] = src_s[starts[b]:starts[b] + n]
            self.eslot[b, :n] = slot_s[starts[b]:starts[b] + n]
        # xr row (within the core's xr table): blk*128 + dst slot; pad -> 0
        dslot = np.where(self.eslot >= 0, self.eslot, 0).astype(np.int64)
        self.exr = np.zeros((nbins, S), np.int64)
        for b in range(nbins):
            self.exr[b] = (b % self.NBLK) * 128 + dslot[b]
        # layer-2 xl row for source j (XL2S layout [group][core][blk%2][slot]
        # where groups pair two blocks per AllGather of h)
        self.l2row = ((blk_of // 2) * (256 * NCORES) + core_of * 256
                      + (blk_of % 2) * 128 + nslot)
        # superchunk structure
        scs = []
        c = self.CBLK
        while c > 0:
            scs.append(min(SCMAX, c))
            c -= min(SCMAX, c)
        self.SCS = scs
        # per-core index/seg tables, wrapped PER SUPERCHUNK so that device
        # slices are contiguous. Layer 1 uses a combined [src | N+xr] list.
        self.idxl = [[None, None] for _ in range(NCORES)]
        self.idxr = [None] * NCORES
        self.seg = [None] * NCORES
        for core in range(NCORES):
            l1, l2, xr, sg = [], [], [], []
            for b in range(self.NBLK):
                g = core * self.NBLK + b
                ch = 0
                for C in self.SCS:
                    sl = slice(ch * 128, (ch + C) * 128)
                    l1.append(_wrap_idxs(np.concatenate(
                        [self.esrc[g][sl], N + self.exr[g][sl]])))
                    l2.append(_wrap_idxs(self.l2row[self.esrc[g][sl]]))
                    xr.append(_wrap_idxs(self.exr[g][sl]))
                    ch += C
                sg.append(self.eslot[g].reshape(self.CBLK, 128).T)
            self.idxl[core] = [np.concatenate(l1, 1), np.concatenate(l2, 1)]
            self.idxr[core] = np.concatenate(xr, 1)
            self.seg[core] = np.concatenate(sg, 1).astype(bf16)  # [128, NBLK*CBLK]
        self.blk_cols1 = self.idxl[0][0].shape[1] // self.NBLK    # combined l1
        self.blk_cols2 = self.idxl[0][1].shape[1] // self.NBLK
        self.xr_cols = self.idxr[0].shape[1] // self.NBLK


# --------------------------------------------------------------- device build

def _build_nc(p):
    N, F_IN, HID, OUT, H = p.N, p.F_IN, p.HID, p.OUT, p.H
    NBLK, CBLK, SCS = p.NBLK, p.CBLK, p.SCS
    XPAD = ((N + 127) // 128) * 128
    NOWN = NBLK * 128
    KIN = F_IN // 128
    K1 = HID // 128   # k-chunks of layer-2 dense input
    KF = {1: HID // 128, 2: OUT // 128}
    NROW2 = NBLK * 128 * NCORES

    nc = bacc.Bacc("TRN2", target_bir_lowering=False, debug=False,
                   num_devices=NCORES)
    # ---- external inputs
    xT = nc.dram_tensor("xT", [F_IN, XPAD], BF, kind="ExternalInput")
    xownT = nc.dram_tensor("xownT", [F_IN, NOWN], BF, kind="ExternalInput")
    WlT1 = nc.dram_tensor("WlT1", [F_IN, HID], BF, kind="ExternalInput")
    WrT1 = nc.dram_tensor("WrT1", [F_IN, HID], BF, kind="ExternalInput")
    WlT2 = nc.dram_tensor("WlT2", [HID, OUT], BF, kind="ExternalInput")
    WrT2 = nc.dram_tensor("WrT2", [HID, OUT], BF, kind="ExternalInput")
    att1 = nc.dram_tensor("att1", [HID, H], BF, kind="ExternalInput")
    att2 = nc.dram_tensor("att2", [OUT, H], BF, kind="ExternalInput")
    b1rep = nc.dram_tensor("b1rep", [128, HID], F32, kind="ExternalInput")
    b2rep = nc.dram_tensor("b2rep", [128, OUT], F32, kind="ExternalInput")
    gam = nc.dram_tensor("gam", [128, OUT], F32, kind="ExternalInput")
    bet = nc.dram_tensor("bet", [128, OUT], F32, kind="ExternalInput")
    iota = nc.dram_tensor("iota", [128, 128], BF, kind="ExternalInput")
    ident = nc.dram_tensor("ident", [128, 128], BF, kind="ExternalInput")
    identf = nc.dram_tensor("identf", [128, 128], F32, kind="ExternalInput")
    blr1 = nc.dram_tensor("blr1", [1, HID], BF, kind="ExternalInput")
    brr1 = nc.dram_tensor("brr1", [1, HID], BF, kind="ExternalInput")
    blr2 = nc.dram_tensor("blr2", [1, OUT], BF, kind="ExternalInput")
    brr2 = nc.dram_tensor("brr2", [1, OUT], BF, kind="ExternalInput")
    idxl1 = nc.dram_tensor("idxl1", list(p.idxl[0][0].shape), I16, kind="ExternalInput")
    idxl2 = nc.dram_tensor("idxl2", list(p.idxl[0][1].shape), I16, kind="ExternalInput")
    idxr = nc.dram_tensor("idxr", list(p.idxr[0].shape), I16, kind="ExternalInput")
    seg = nc.dram_tensor("seg", list(p.seg[0].shape), BF, kind="ExternalInput")
    # ---- outputs
    out_o = nc.dram_tensor("out_o", [NOWN, OUT], F32, kind="ExternalOutput")
    # ---- internal dram
    NG = (NBLK + 1) // 2  # block pairs per h-AllGather
    XLR1 = nc.dram_tensor("XLR1", [N + NOWN, HID], BF)
    xr2tab = nc.dram_tensor("xr2tab", [NOWN, OUT], BF)
    hownT = nc.dram_tensor("hownT", [NG, HID, 256], BF)
    HST = nc.dram_tensor("HST", [NG * NCORES * HID, 256], BF,
                         addr_space="Shared")
    XL2S = nc.dram_tensor("XL2S", [NG * NCORES * 256, OUT], BF)
    warm_in = nc.dram_tensor("warm_in", [1, 64], F32)
    warm_out = nc.dram_tensor("warm_out", [1, 64], F32, addr_space="Shared")

    with tile.TileContext(nc) as tc:
        with (
            tc.tile_pool(name="cons", bufs=1) as cons,
            tc.tile_pool(name="ps_dense", bufs=1, space="PSUM") as psd_pool,
        ):
            # comm-path warmup: tiny AllReduce issued first so the
            # one-time collective initialization overlaps dense/edge compute
            wt = cons.tile([1, 64], F32, tag="warm")
            nc.vector.memset(wt[:], 1.0)
            nc.sync.dma_start(warm_in[:], wt[:])
            nc.gpsimd.collective_compute(
                "AllReduce", mybir.AluOpType.add,
                replica_groups=[list(range(NCORES))],
                ins=[warm_in[:].opt()], outs=[warm_out[:].opt()])
            # ---------------- constants
            wl1 = cons.tile([128, KIN, HID], BF)
            wr1 = cons.tile([128, KIN, HID], BF)
            wl2 = cons.tile([128, K1, OUT], BF)
            wr2 = cons.tile([128, K1, OUT], BF)
            for k in range(KIN):
                nc.sync.dma_start(wl1[:, k, :], WlT1[k * 128:(k + 1) * 128, :])
                nc.sync.dma_start(wr1[:, k, :], WrT1[k * 128:(k + 1) * 128, :])
            for k in range(K1):
                nc.sync.dma_start(wl2[:, k, :], WlT2[k * 128:(k + 1) * 128, :])
                nc.sync.dma_start(wr2[:, k, :], WrT2[k * 128:(k + 1) * 128, :])
            att_t = {}
            for lay, (attd, Fo) in {1: (att1, HID), 2: (att2, OUT)}.items():
                a = cons.tile([128, Fo // 128, H], BF, tag=f"att{lay}")
                for k in range(Fo // 128):
                    nc.sync.dma_start(a[:, k, :], attd[k * 128:(k + 1) * 128, :])
                att_t[lay] = a
            b1rep_t = cons.tile([128, HID], F32)
            b2rep_t = cons.tile([128, OUT], F32)
            gam_t = cons.tile([128, OUT], F32)
            bet_t = cons.tile([128, OUT], F32)
            iota_t = cons.tile([128, 128], BF)
            id_t = cons.tile([128, 128], BF)
            idf_t = cons.tile([128, 128], F32)
            for t, d in [(b1rep_t, b1rep), (b2rep_t, b2rep), (gam_t, gam),
                         (bet_t, bet), (iota_t, iota), (id_t, ident),
                         (idf_t, identf)]:
                nc.sync.dma_start(t[:], d[:])
            ones_t = cons.tile([1, 128], BF)
            nc.vector.memset(ones_t[:], 1.0)
            eps_t = cons.tile([128, 1], F32)
            nc.vector.memset(eps_t[:], EPS)
            blr1_t = cons.tile([1, HID], BF, tag="blr1")
            brr1_t = cons.tile([1, HID], BF, tag="brr1")
            blr2_t = cons.tile([1, OUT], BF, tag="blr2")
            brr2_t = cons.tile([1, OUT], BF, tag="brr2")
            for t, d in [(blr1_t, blr1), (brr1_t, brr1), (blr2_t, blr2),
                         (brr2_t, brr2)]:
                nc.sync.dma_start(t[:], d[:])
            idxl1_t = cons.tile(list(p.idxl[0][0].shape), I16)
            idxl2_t = cons.tile(list(p.idxl[0][1].shape), I16)
            idxr_t = cons.tile(list(p.idxr[0].shape), I16)
            seg_t = cons.tile(list(p.seg[0].shape), BF)
            nc.sync.dma_start(idxl1_t[:], idxl1[:])
            nc.sync.dma_start(idxl2_t[:], idxl2[:])
            nc.sync.dma_start(idxr_t[:], idxr[:])
            nc.sync.dma_start(seg_t[:], seg[:])

            # ---------------- dense layer 1 (replicated)
            with tc.tile_pool(name="d1", bufs=1) as d1p, \
                 tc.tile_pool(name="d1w", bufs=3) as d1w:
                xT_t = d1p.tile([128, KIN, XPAD], BF)
                for k in range(KIN):
                    nc.sync.dma_start(xT_t[:, k, :], xT[k * 128:(k + 1) * 128, :])
                xoT_t = d1p.tile([128, KIN, NOWN], BF)
                for k in range(KIN):
                    nc.sync.dma_start(xoT_t[:, k, :], xownT[k * 128:(k + 1) * 128, :])

                def dense(lhsT, w_t, kk, Fo, rows, dst, dst_off, tag, brow):
                    ps = psd_pool.tile([128, Fo], F32, tag="dense")
                    for k in range(kk):
                        nc.tensor.matmul(ps[:], lhsT[:, k, :], w_t[:, k, :],
                                         start=(k == 0),
                                         stop=(k == kk - 1 and brow is None))
                    if brow is not None:
                        nc.tensor.matmul(ps[:], ones_t[:], brow[:],
                                         start=False, stop=True)
                    o = d1w.tile([128, Fo], BF, tag=tag)
                    nc.scalar.copy(o[:], ps[:])
                    nc.sync.dma_start(dst[dst_off:dst_off + rows, :], o[:rows, :])

                bl1b = blr1_t if p.use_bias else None
                br1b = brr1_t if p.use_bias else None
                for ch in range(XPAD // 128):
                    rows = min(128, N - ch * 128)
                    if rows <= 0:
                        break
                    lhsT = xT_t[:, :, ch * 128:ch * 128 + 128]
                    dense(lhsT, wl1, KIN, HID, rows, XLR1, ch * 128, "d1o", bl1b)
                for b in range(NBLK):
                    lhsT = xoT_t[:, :, b * 128:(b + 1) * 128]
                    dense(lhsT, wr1, KIN, HID, 128, XLR1, N + b * 128, "d1o", br1b)

            # ---------------- edge phases (layer-scoped pools)
            def emit_edge_block(pools, lay, b, Fo):
                (g_pool, ew_pool, eo_pool, pst_pool, psv_pool, psdn_pool,
                 pse_pool, pss_pool) = pools
                kf = Fo // 128
                psv = psv_pool.tile([128, Fo], F32, tag="aggv")
                psd = psdn_pool.tile([4, 128], F32, tag="aggd")
                ch = 0
                off1 = 0  # running col offset of combined l1 idx within block
                off2 = 0
                for C in SCS:
                    if lay == 1:
                        cols0 = b * p.blk_cols1 + off1
                        g = g_pool.tile([128, 2 * SCMAX, Fo], BF, tag="g1")
                        nc.gpsimd.dma_gather(
                            g[:, :2 * C, :], XLR1[:],
                            idxl1_t[:, cols0:cols0 + 2 * C * 8],
                            2 * C * 128, 2 * C * 128, Fo)
                        xlg = g[:, :C, :]
                        xrg = g[:, C:2 * C, :]
                    else:
                        cols0 = b * p.blk_cols2 + off2
                        colsr = b * p.xr_cols + off2
                        gl = g_pool.tile([128, SCMAX, Fo], BF, tag="gl2")
                        nc.gpsimd.dma_gather(
                            gl[:, :C, :], XL2S[:],
                            idxl2_t[:, cols0:cols0 + C * 8],
                            C * 128, C * 128, Fo)
                        gr = g_pool.tile([128, SCMAX, Fo], BF, tag="gr2")
                        nc.gpsimd.dma_gather(
                            gr[:, :C, :], xr2tab[:],
                            idxr_t[:, colsr:colsr + C * 8],
                            C * 128, C * 128, Fo)
                        xlg = gl[:, :C, :]
                        xrg = gr[:, :C, :]
                    off1 += 2 * C * 8
                    off2 += C * 8
                    s = ew_pool.tile([128, SCMAX, Fo], BF, tag="s")
                    nc.vector.tensor_tensor(s[:, :C, :], xlg, xrg,
                                            op=mybir.AluOpType.add)
                    t = ew_pool.tile([128, SCMAX, Fo], BF, tag="t")
                    nc.vector.scalar_tensor_tensor(
                        t[:, :C, :], s[:, :C, :], SLOPE, s[:, :C, :],
                        op0=mybir.AluOpType.mult, op1=mybir.AluOpType.max)
                    # PE transposes of t chunks into PSUM (bank groups of 8)
                    ntp = C * kf
                    tT_ps = pst_pool.tile([128, SCMAX * kf * 128], BF, tag="tT")
                    for i in range(ntp):
                        c, k = divmod(i, kf)
                        nc.tensor.matmul(
                            tT_ps[:, i * 128:(i + 1) * 128],
                            t[:, c, k * 128:(k + 1) * 128], id_t[:],
                            is_transpose=True,
                            start=(i % 8 == 0),
                            stop=(i % 8 == 7) or (i == ntp - 1))
                    tT = ew_pool.tile([128, SCMAX * kf * 128], BF, tag="tTs")
                    nc.scalar.copy(tT[:, :ntp * 128], tT_ps[:, :ntp * 128])
                    # dot with att: e[h, c*128+e'] accumulated over kf
                    e_ps = pse_pool.tile([4, SCMAX * 128], F32, tag="e")
                    tT3 = tT[:, :ntp * 128].rearrange(
                        "p (c k e) -> p c k e", c=C, k=kf)
                    for k in range(kf):
                        nc.tensor.matmul(
                            e_ps[:, :C * 128].rearrange("h (c e) -> h c e", c=C),
                            att_t[lay][:, k, :], tT3[:, :, k, :],
                            start=(k == 0), stop=(k == kf - 1))
                    wT = ew_pool.tile([4, SCMAX * 128], BF, tag="wT")
                    nc.scalar.activation(wT[:, :C * 128], e_ps[:, :C * 128],
                                         mybir.ActivationFunctionType.Exp)
                    wn_ps = pss_pool.tile([128, SCMAX * 4], BF, tag="small")
                    for c in range(C):
                        nc.tensor.matmul(
                            wn_ps[:, c * 4:(c + 1) * 4],
                            wT[:, c * 128:(c + 1) * 128], id_t[:4, :4],
                            is_transpose=True,
                            start=(c == 0), stop=(c == C - 1))
                    wn = ew_pool.tile([128, SCMAX, 4], BF, tag="wns")
                    nc.vector.tensor_copy(
                        wn[:, :C, :].rearrange("p c h -> p (c h)"),
                        wn_ps[:, :C * 4])
                    oh = ew_pool.tile([128, SCMAX, 128], BF, tag="oh")
                    nc.vector.tensor_tensor(
                        out=oh[:, :C, :],
                        in0=iota_t[:].unsqueeze(1).to_broadcast((128, C, 128)),
                        in1=seg_t[:, b * CBLK + ch:b * CBLK + ch + C]
                            .unsqueeze(2).to_broadcast((128, C, 128)),
                        op=mybir.AluOpType.is_equal)
                    v = ew_pool.tile([128, SCMAX, Fo], BF, tag="v")
                    if Fo // H >= 128:
                        # per-(chunk,head) tensor_scalar hits the 4x DVE mode
                        wnf = ew_pool.tile([128, SCMAX, 4], F32, tag="wnf")
                        nc.vector.tensor_copy(
                            wnf[:, :C, :].rearrange("p c h -> p (c h)"),
                            wn_ps[:, :C * 4])
                        d_ = Fo // H
                        for c in range(C):
                            for hh in range(H):
                                nc.vector.tensor_scalar_mul(
                                    v[:, c, hh * d_:(hh + 1) * d_],
                                    xlg[:, c, hh * d_:(hh + 1) * d_],
                                    wnf[:, c, hh:hh + 1])
                    else:
                        nc.vector.tensor_tensor(
                            out=v[:, :C, :].rearrange("p c (h d) -> p c h d", h=H),
                            in0=xlg.rearrange("p c (h d) -> p c h d", h=H),
                            in1=wn[:, :C, :].unsqueeze(3).to_broadcast(
                                (128, C, H, Fo // H)),
                            op=mybir.AluOpType.mult)
                    for c in range(C):
                        cc = ch + c
                        nc.tensor.matmul(psv[:], oh[:, c, :], v[:, c, :],
                                         start=(cc == 0), stop=(cc == CBLK - 1))
                        nc.tensor.matmul(psd[:], wn[:, c, :], oh[:, c, :],
                                         start=(cc == 0), stop=(cc == CBLK - 1))
                    ch += C
                # ---- epilogue: normalize by denominators
                denT = ew_pool.tile([4, 128], F32, tag="denT")
                nc.vector.tensor_scalar_add(denT[:], psd[:], 1e-30)
                rec = ew_pool.tile([4, 128], F32, tag="rec")
                nc.vector.reciprocal(rec[:], denT[:])
                rec_ps = pss_pool.tile([128, 4], F32, tag="small")
                nc.tensor.matmul(rec_ps[:], rec[:], idf_t[:4, :4],
                                 is_transpose=True)
                recn = ew_pool.tile([128, 4], F32, tag="recn")
                nc.vector.tensor_copy(recn[:], rec_ps[:])
                vn = eo_pool.tile([128, Fo], F32, tag="vn")
                nc.vector.tensor_tensor(
                    out=vn[:].rearrange("p (h d) -> p h d", h=H),
                    in0=psv[:].rearrange("p (h d) -> p h d", h=H),
                    in1=recn[:].unsqueeze(2).to_broadcast((128, H, Fo // H)),
                    op=mybir.AluOpType.mult)
                return vn

            with (
                tc.tile_pool(name="ps_v", bufs=2, space="PSUM") as psv_pool,
                tc.tile_pool(name="ps_d", bufs=1, space="PSUM") as psdn_pool,
                tc.tile_pool(name="ps_e", bufs=1, space="PSUM") as pse_pool,
                tc.tile_pool(name="ps_s", bufs=1, space="PSUM") as pss_pool,
            ):
                psum_pools = (psv_pool, psdn_pool, pse_pool, pss_pool)
                # ======== layer 1 blocks + dense layer 2 + allgather
                with (
                    tc.tile_pool(name="g1", bufs=4) as g_pool,
                    tc.tile_pool(name="ew1", bufs=4) as ew_pool,
                    tc.tile_pool(name="eo1", bufs=2) as eo_pool,
                    tc.tile_pool(name="ps_t1", bufs=2, space="PSUM") as pst_pool,
                ):
                    pools = (g_pool, ew_pool, eo_pool, pst_pool) + psum_pools

                    def dense_xl2_group(g):
                        """xl2 for group g's gathered h rows (all cores)."""
                        gsz = min(2, NBLK - g * 2)
                        for core in range(NCORES):
                            htt = eo_pool.tile([128, K1, 256], BF, tag="htt")
                            for k in range(K1):
                                r0 = (g * NCORES + core) * HID + k * 128
                                nc.sync.dma_start(
                                    htt[:, k, :gsz * 128],
                                    HST[r0:r0 + 128, :gsz * 128])
                            for j in range(gsz):
                                ps = psd_pool.tile([128, OUT], F32, tag="dense")
                                for k in range(K1):
                                    nc.tensor.matmul(
                                        ps[:], htt[:, k, j * 128:(j + 1) * 128],
                                        wl2[:, k, :], start=(k == 0),
                                        stop=(k == K1 - 1 and not p.use_bias))
                                if p.use_bias:
                                    nc.tensor.matmul(ps[:], ones_t[:], blr2_t[:],
                                                     start=False, stop=True)
                                o = eo_pool.tile([128, OUT], BF, tag="d2o")
                                nc.vector.tensor_copy(o[:], ps[:])
                                base = (g * NCORES + core) * 256 + j * 128
                                nc.sync.dma_start(XL2S[base:base + 128, :], o[:])

                    for b in range(NBLK):
                        vn = emit_edge_block(pools, 1, b, HID)
                        vb = eo_pool.tile([128, HID], F32, tag="vb1")
                        nc.vector.tensor_tensor(vb[:], vn[:], b1rep_t[:],
                                                op=mybir.AluOpType.add)
                        h = eo_pool.tile([128, HID], BF, tag="h1")
                        nc.scalar.activation(h[:], vb[:],
                                             mybir.ActivationFunctionType.Relu)
                        # transposed h for this block -> hownT + xr2 dense
                        hT_ps = pss_pool.tile([128, HID], BF, tag="small")
                        for k in range(K1):
                            nc.tensor.matmul(hT_ps[:, k * 128:(k + 1) * 128],
                                             h[:, k * 128:(k + 1) * 128], id_t[:],
                                             is_transpose=True,
                                             start=(k == 0), stop=(k == K1 - 1))
                        hT = eo_pool.tile([128, HID], BF, tag="hTs")
                        nc.scalar.copy(hT[:], hT_ps[:])
                        for k in range(K1):
                            nc.sync.dma_start(
                                hownT[b // 2, k * 128:(k + 1) * 128,
                                      (b % 2) * 128:(b % 2 + 1) * 128],
                                hT[:, k * 128:(k + 1) * 128])
                        ps = psd_pool.tile([128, OUT], F32, tag="dense")
                        for k in range(K1):
                            nc.tensor.matmul(ps[:], hT[:, k * 128:(k + 1) * 128],
                                             wr2[:, k, :], start=(k == 0),
                                             stop=(k == K1 - 1 and p.use_bias is False))
                        if p.use_bias:
                            nc.tensor.matmul(ps[:], ones_t[:], brr2_t[:],
                                             start=False, stop=True)
                        o = eo_pool.tile([128, OUT], BF, tag="d2o")
                        nc.scalar.copy(o[:], ps[:])
                        nc.sync.dma_start(xr2tab[b * 128:(b + 1) * 128, :], o[:])
                        # AllGather transposed h per pair of blocks
                        if b % 2 == 1 or b == NBLK - 1:
                            g = b // 2
                            nc.gpsimd.collective_compute(
                                "AllGather", mybir.AluOpType.bypass,
                                replica_groups=[list(range(NCORES))],
                                ins=[hownT[g].opt()],
                                outs=[HST[g * NCORES * HID:(g + 1) * NCORES * HID,
                                          :].opt()])
                        # overlap: xl2 dense for the PREVIOUS group
                        if b % 2 == 1 and b >= 3:
                            dense_xl2_group(b // 2 - 1)
                    dense_xl2_group(NG - 1)

                # ======== layer 2 blocks + layernorm
                with (
                    tc.tile_pool(name="g2", bufs=4) as g_pool,
                    tc.tile_pool(name="ew2", bufs=4) as ew_pool,
                    tc.tile_pool(name="eo2", bufs=2) as eo_pool,
                    tc.tile_pool(name="ps_t2", bufs=1, space="PSUM") as pst_pool,
                ):
                    pools = (g_pool, ew_pool, eo_pool, pst_pool) + psum_pools
                    for b in range(NBLK):
                        vn = emit_edge_block(pools, 2, b, OUT)
                        vb = eo_pool.tile([128, OUT], F32, tag="vb2")
                        nc.vector.tensor_tensor(vb[:], vn[:], b2rep_t[:],
                                                op=mybir.AluOpType.add)
                        # layernorm over OUT
                        tmp = eo_pool.tile([128, OUT], F32, tag="lntmp")
                        ssum = eo_pool.tile([128, 1], F32, tag="lnsum")
                        nc.scalar.activation(tmp[:], vb[:],
                                             mybir.ActivationFunctionType.Copy,
                                             accum_out=ssum[:])
                        negmu = eo_pool.tile([128, 1], F32, tag="lnmu")
                        nc.vector.tensor_scalar_mul(negmu[:], ssum[:], -1.0 / OUT)
                        xm = eo_pool.tile([128, OUT], F32, tag="lnxm")
                        nc.scalar.activation(xm[:], vb[:],
                                             mybir.ActivationFunctionType.Identity,
                                             bias=negmu[:])
                        sq = eo_pool.tile([128, OUT], F32, tag="lnsq")
                        ssq = eo_pool.tile([128, 1], F32, tag="lnssq")
                        nc.scalar.activation(sq[:], xm[:],
                                             mybir.ActivationFunctionType.Square,
                                             accum_out=ssq[:])
                        sd = eo_pool.tile([128, 1], F32, tag="lnsd")
                        nc.scalar.activation(sd[:], ssq[:],
                                             mybir.ActivationFunctionType.Sqrt,
                                             scale=1.0 / OUT, bias=eps_t[:])
                        rstd = eo_pool.tile([128, 1], F32, tag="lnrstd")
                        nc.vector.reciprocal(rstd[:], sd[:])
                        og = eo_pool.tile([128, OUT], F32, tag="lnog")
                        nc.vector.scalar_tensor_tensor(
                            og[:], xm[:], rstd[:], gam_t[:],
                            op0=mybir.AluOpType.mult, op1=mybir.AluOpType.mult)
                        ob = eo_pool.tile([128, OUT], F32, tag="lnob")
                        nc.vector.tensor_tensor(ob[:], og[:], bet_t[:],
                                                op=mybir.AluOpType.add)
                        nc.sync.dma_start(out_o[b * 128:(b + 1) * 128, :], ob[:])

    nc.compile()
    return nc


# --------------------------------------------------------------- input maps

def _make_in_maps(p, inputs):
    N, F_IN, HID, OUT, H = p.N, p.F_IN, p.HID, p.OUT, p.H
    XPAD = ((N + 127) // 128) * 128
    NOWN = p.NBLK * 128
    x = np.asarray(inputs["x"], np.float32)
    xpad = np.zeros((XPAD, F_IN), np.float32)
    xpad[:N] = x
    xT = np.ascontiguousarray(xpad.T).astype(bf16)

    def attfull(att, Fo):
        H_, d = att.shape
        a = np.zeros((Fo, H_), np.float32)
        for h in range(H_):
            a[h * d:(h + 1) * d, h] = att[h]
        return a.astype(bf16)

    Wl1 = np.asarray(inputs["Wl1"], np.float32)
    Wr1 = np.asarray(inputs["Wr1"], np.float32)
    Wl2 = np.asarray(inputs["Wl2"], np.float32)
    Wr2 = np.asarray(inputs["Wr2"], np.float32)
    common = dict(
        xT=xT,
        WlT1=np.ascontiguousarray(Wl1.T).astype(bf16),
        WrT1=np.ascontiguousarray(Wr1.T).astype(bf16),
        WlT2=np.ascontiguousarray(Wl2.T).astype(bf16),
        WrT2=np.ascontiguousarray(Wr2.T).astype(bf16),
        att1=attfull(np.asarray(inputs["att1"], np.float32), HID),
        att2=attfull(np.asarray(inputs["att2"], np.float32), OUT),
        b1rep=np.broadcast_to(
            np.asarray(inputs["bias1"], np.float32), (128, HID)).copy(),
        b2rep=np.broadcast_to(
            np.asarray(inputs["bias2"], np.float32), (128, OUT)).copy(),
        gam=np.broadcast_to(
            np.asarray(inputs["gamma"], np.float32), (128, OUT)).copy(),
        bet=np.broadcast_to(
            np.asarray(inputs["beta"], np.float32), (128, OUT)).copy(),
        iota=np.broadcast_to(
            np.arange(128, dtype=np.float32), (128, 128)).astype(bf16),
        ident=np.eye(128, dtype=np.float32).astype(bf16),
        identf=np.eye(128, dtype=np.float32),
    )
    common["blr1"] = np.asarray(inputs["bl1"], np.float32).reshape(1, HID).astype(bf16)
    common["brr1"] = np.asarray(inputs["br1"], np.float32).reshape(1, HID).astype(bf16)
    common["blr2"] = np.asarray(inputs["bl2"], np.float32).reshape(1, OUT).astype(bf16)
    common["brr2"] = np.asarray(inputs["br2"], np.float32).reshape(1, OUT).astype(bf16)
    in_maps = []
    for c in range(NCORES):
        m = dict(common)
        # per-core x_own columns (slot order)
        xo = np.zeros((NOWN, F_IN), np.float32)
        for b in range(p.NBLK):
            g = c * p.NBLK + b
            nodes = np.nonzero(p.nbin == g)[0]
            xo[b * 128 + p.nslot[nodes]] = x[nodes]
        m["xownT"] = np.ascontiguousarray(xo.T).astype(bf16)
        m["idxl1"] = p.idxl[c][0]
        m["idxl2"] = p.idxl[c][1]
        m["idxr"] = p.idxr[c]
        m["seg"] = p.seg[c]
        in_maps.append(m)
    return in_maps


# ----------------------------------------------------------------- runner

class _Runner:
    def __init__(self, inputs):
        ei = np.asarray(inputs["edge_index"])
        N, F_IN = np.asarray(inputs["x"]).shape
        HID = np.asarray(inputs["Wl1"]).shape[0]
        OUT = np.asarray(inputs["Wl2"]).shape[0]
        H = np.asarray(inputs["att1"]).shape[0]
        self.eihash = hash(ei.tobytes())
        self.p = _Prep(N, ei.shape[1], F_IN, HID, OUT, H, ei)
        self.p.use_bias = any(
            np.abs(np.asarray(inputs[k])).max() > 0
            for k in ("bl1", "br1", "bl2", "br2"))
        self.nc = _build_nc(self.p)
        self.jit_fn = None

    def _prep_jit(self):
        """Build the shard_map jit once (mirrors bass2jax.run_bass_via_pjrt)."""
        import jax
        from jax.sharding import Mesh, PartitionSpec
        from jax.experimental.shard_map import shard_map
        from concourse import bass2jax
        from concourse.bass2jax import _bass_exec_p, partition_id_tensor
        nc = self.nc
        bass2jax.install_neuronx_cc_hook()
        pname = nc.partition_id_tensor.name if nc.partition_id_tensor else None
        in_names, out_names, out_avals, zero_outs = [], [], [], []
        for alloc in nc.m.functions[0].allocations:
            if not isinstance(alloc, mybir.MemoryLocationSet):
                continue
            name = alloc.memorylocations[0].name
            if alloc.kind == "ExternalInput":
                if name != pname:
                    in_names.append(name)
            elif alloc.kind == "ExternalOutput":
                out_names.append(name)
                shape = tuple(alloc.tensor_shape)
                dtype = mybir.dt.np(alloc.dtype)
                out_avals.append(jax.core.ShapedArray(shape, dtype))
                zero_outs.append(np.zeros(shape, dtype))
        n_params = len(in_names)
        all_names = in_names + out_names
        if pname is not None:
            all_names = all_names + [pname]

        def _body(*args):
            operands = list(args)
            if pname is not None:
                operands.append(partition_id_tensor())
            outs = _bass_exec_p.bind(
                *operands, out_avals=tuple(out_avals), in_names=tuple(all_names),
                out_names=tuple(out_names), lowering_input_output_aliases=(),
                sim_require_finite=True, sim_require_nnan=True, nc=nc)
            return tuple(outs)

        devices = jax.devices()[:NCORES]
        mesh = Mesh(np.asarray(devices), ("core",))
        n_outs = len(out_names)
        self.jit_fn = jax.jit(
            shard_map(_body, mesh=mesh,
                      in_specs=(PartitionSpec("core"),) * (n_params + n_outs),
                      out_specs=(PartitionSpec("core"),) * n_outs,
                      check_rep=False),
            keep_unused=True)
        self.in_names = in_names
        self.out_names = out_names
        self.out_avals = out_avals
        self.zero_outs = zero_outs
        self.mesh = mesh

    def device_args(self, inputs):
        in_maps = _make_in_maps(self.p, inputs)
        concat_in = [np.concatenate([in_maps[c][n] for c in range(NCORES)], 0)
                     for n in self.in_names]
        concat_zero = [np.zeros((NCORES * z.shape[0], *z.shape[1:]), z.dtype)
                       for z in self.zero_outs]
        return concat_in + concat_zero

    def run(self, inputs):
        if self.jit_fn is None:
            self._prep_jit()
        args = self.device_args(inputs)
        out_arrs = self.jit_fn(*args)
        res = [
            {n: np.asarray(out_arrs[i]).reshape(
                NCORES, *self.out_avals[i].shape)[c]
             for i, n in enumerate(self.out_names)}
            for c in range(NCORES)
        ]
        return self.assemble(res)

    def assemble(self, res):
        p = self.p
        out = np.zeros((p.N, p.OUT), np.float32)
        for c in range(NCORES):
            o = np.asarray(res[c]["out_o"], np.float32)
            for b in range(p.NBLK):
                g = c * p.NBLK + b
                nodes = np.nonzero(p.nbin == g)[0]
                out[nodes] = o[b * 128 + p.nslot[nodes]]
        return out

    def timed_loop(self, inputs, r1=4, r2=40, reps=2):
        """Async-pipelined dispatch timing; difference two batch sizes to
        cancel fixed per-batch overhead."""
        import jax
        from jax.sharding import NamedSharding, PartitionSpec
        if self.jit_fn is None:
            self._prep_jit()
        args = self.device_args(inputs)
        sh = NamedSharding(self.mesh, PartitionSpec("core"))
        dargs = [jax.device_put(a, sh) for a in args]
        jax.block_until_ready(dargs)
        out = self.jit_fn(*dargs)
        jax.block_until_ready(out)

        def batch(R):
            ts = []
            for _ in range(reps):
                t0 = time.perf_counter()
                outs = [self.jit_fn(*dargs) for _ in range(R)]
                jax.block_until_ready(outs)
                ts.append(time.perf_counter() - t0)
            return min(ts)

        t1, t2 = batch(r1), batch(r2)
        return (t2 - t1) / (r2 - r1) * 1e9

    def timed(self, inputs, reps=5):
        import jax
        from jax.sharding import NamedSharding, PartitionSpec
        if self.jit_fn is None:
            self._prep_jit()
        args = self.device_args(inputs)
        sh = NamedSharding(self.mesh, PartitionSpec("core"))
        dargs = [jax.device_put(a, sh) for a in args]
        jax.block_until_ready(dargs)
        out = self.jit_fn(*dargs)
        jax.block_until_ready(out)
        times = []
        for _ in range(reps):
            t0 = time.perf_counter()
            out = self.jit_fn(*dargs)
            jax.block_until_ready(out)
            times.append(time.perf_counter() - t0)
        return min(times) * 1e9


_CACHE = {}


def kernel(**inputs):
    ei = np.asarray(inputs["edge_index"])
    key = hash(ei.tobytes())
    if key not in _CACHE:
        _CACHE.clear()
        _CACHE[key] = _Runner(inputs)
    r = _CACHE[key]
    try:
        return r.run(inputs)
    except Exception:
        # fallback: the plain concourse SPMD runner
        from concourse.bass_utils import run_bass_kernel_spmd
        in_maps = _make_in_maps(r.p, inputs)
        res = run_bass_kernel_spmd(r.nc, in_maps, list(range(NCORES)))
        return r.assemble(res.results)

